# revision 6
# baseline (speedup 1.0000x reference)
"""Trainium2 Bass kernel for nn_CUDAOptimizedBKCore: diagonal Green's function
of a complex-shifted tridiagonal matrix via forward/backward continuant
recursions (theta/phi), data-parallel over the batch across 8 NeuronCores.

Self-contained: takes FULL inputs, shards B across cores, runs the Bass
program via run_bass_kernel_spmd, gathers the FULL output.

Per-core design (v2):
  - The per-partition row dim f is split into two engine-private slices
    (DVE: FV rows, Pool: FP rows). Each engine runs the entire pipeline
    (theta scan -> w -> phi scan + combines) on its own slice with no
    cross-engine dependencies, so both engines stay throughput-bound.
  - DVE scan step is 3 instructions: one tensor_tensor multiply
    (a (x) c_swap, with the +-a sign pair prefolded into the A2 tile) and
    two TensorScalarPtr-class ops, which run at 2 elem/cycle fp32 on DVE.
    Pool has no TensorScalarPtr support, so its step is plain TTs (3 when
    the off-diagonal product s == 1, which holds for these inputs).
  - theta history tile TH is laid out [P, f, n, 2] so each combine's G row
    overwrites the theta row it just consumed; at the end TH *is* the
    output in (row, k, comp) order: per-partition contiguous 640B rows,
    so the final DMA moves at full HBM rate.
"""
import numpy as np

import concourse.bass as bass
import concourse.bacc as bacc
import concourse.tile as tile
from concourse import mybir

F32 = mybir.dt.float32
P = 128
RING = 16
CB = 8          # combine batch (phi'' values per batched combine)

_CACHE = {}


def build_nc(b_core: int, n: int, f: int, n_cores: int = 8, loops: int = 1,
             s_one: bool = True):
    """Build the Bacc program for one core's slice (b_core rows, n steps)."""
    assert b_core == P * f
    assert n % CB == 0 and RING % CB == 0
    fv = (3 * f) // 4           # DVE row slice
    fp = f - fv                 # Pool row slice
    nc = bacc.Bacc("TRN2", target_bir_lowering=False, debug=False, num_devices=n_cores)
    he = nc.dram_tensor("he", [b_core, n], F32, kind="ExternalInput").ap()
    dvec = nc.dram_tensor("dvec", [P, n], F32, kind="ExternalInput").ap()
    svf = nc.dram_tensor("svf", [P, n], F32, kind="ExternalInput").ap()
    svb = nc.dram_tensor("svb", [P, n], F32, kind="ExternalInput").ap()
    g = nc.dram_tensor("g", [b_core, 2 * n], F32, kind="ExternalOutput").ap()

    mult, add, sub = mybir.AluOpType.mult, mybir.AluOpType.add, mybir.AluOpType.subtract
    he3 = he.rearrange("(p f) k -> p f k", p=P)
    g4 = g.rearrange("(p f) (k c) -> p f k c", p=P, c=2)

    with tile.TileContext(nc) as tc:
        with (
            tc.tile_pool(name="aux", bufs=1) as aux,
            tc.tile_pool(name="big", bufs=1) as big,
            tc.tile_pool(name="tmpv", bufs=1) as tmpv,
            tc.tile_pool(name="tmpp", bufs=1) as tmpp,
            tc.tile_pool(name="qv", bufs=1) as qvp,
            tc.tile_pool(name="qp", bufs=1) as qpp,
        ):
            d_t = aux.tile([P, n], F32)
            nc.sync.dma_start(out=d_t[:], in_=dvec)
            sf_t = aux.tile([P, n], F32)
            nc.sync.dma_start(out=sf_t[:], in_=svf)
            sb_t = aux.tile([P, n], F32)
            nc.sync.dma_start(out=sb_t[:], in_=svb)
            neg1 = aux.tile([P, 1], F32)
            nc.gpsimd.memset(neg1[:], -1.0)

            import contextlib
            loop_cm = tc.For_i(0, loops, 1) if loops > 1 else contextlib.nullcontext()
            with loop_cm:
                # Per-engine state tiles. A2[:, 1] = a = he + d, A2[:, 0] = -a.
                A2 = {}
                TH = {}
                THn = {}
                ring = {}
                for key, fe in (("v", fv), ("p", fp)):
                    A2[key] = big.tile([P, 2, fe, n], F32, name=f"A2{key}")
                    TH[key] = big.tile([P, fe, n, 2], F32, name=f"TH{key}")
                    THn[key] = big.tile([P, 2, fe], F32, name=f"THn{key}")
                    ring[key] = big.tile([P, RING, 2, fe], F32, name=f"ring{key}")

                j0 = {"v": 0, "p": fv}
                eng = {"v": nc.vector, "p": nc.gpsimd}
                tmp = {"v": tmpv, "p": tmpp}
                qpool = {"v": qvp, "p": qpp}
                fe_of = {"v": fv, "p": fp}

                # he slices straight into the A2 "+a" plane (contiguous both
                # sides), then a = he + d and -a built in place per engine.
                for key in ("v", "p"):
                    fe, j = fe_of[key], j0[key]
                    e = eng[key]
                    nc.sync.dma_start(out=A2[key][:, 1], in_=he3[:, j:j + fe])
                    d_b = d_t.unsqueeze(1).broadcast_to([P, fe, n])
                    if key == "v":
                        e.scalar_tensor_tensor(
                            out=A2[key][:, 1], in0=A2[key][:, 1], scalar=1.0,
                            in1=d_b, op0=mult, op1=add,
                        )
                        e.tensor_scalar_mul(A2[key][:, 0], A2[key][:, 1], -1.0)
                    else:
                        e.tensor_add(out=A2[key][:, 1], in0=A2[key][:, 1], in1=d_b)
                        n1_b = neg1.unsqueeze(1).broadcast_to([P, fe, n])
                        e.tensor_tensor(
                            out=A2[key][:, 0], in0=A2[key][:, 1], in1=n1_b, op=mult,
                        )

                def th_slot(key, k):
                    # theta'_k row as [P, 2, fe]
                    return TH[key][:, :, k, :].transpose([0, 2, 1])

                def th_slot_swap(key, k):
                    return TH[key][:, :, k, ::-1].transpose([0, 2, 1])

                # init theta'_0 = (1, 0); theta'_1 = (1, a_0)
                for key in ("v", "p"):
                    e = eng[key]
                    e.memset(TH[key][:, :, 0, 0], 1.0)
                    e.memset(TH[key][:, :, 0, 1], 0.0)
                    e.memset(TH[key][:, :, 1, 0], 1.0)
                    e.tensor_copy(TH[key][:, :, 1, 1], A2[key][:, 1, :, 0])

                def advance(key, a_k, c, c_swap, pprev, out, s_t, tag):
                    """out = (c + s*p) + A2[.., a_k] (x) c_swap, one engine."""
                    fe = fe_of[key]
                    e = eng[key]
                    m = tmp[key].tile([P, 2, fe], F32, tag=f"m{key}", name=f"m_{key}{tag}")
                    e.tensor_tensor(out=m[:], in0=A2[key][:, :, :, a_k], in1=c_swap, op=mult)
                    if key == "v":
                        u = tmp[key].tile([P, 2, fe], F32, tag=f"u{key}", name=f"u_{key}{tag}")
                        e.scalar_tensor_tensor(
                            out=u[:], in0=pprev, scalar=s_t, in1=c, op0=mult, op1=add,
                        )
                        e.scalar_tensor_tensor(
                            out=out, in0=m[:], scalar=1.0, in1=u[:], op0=mult, op1=add,
                        )
                    else:
                        u = tmp[key].tile([P, 2, fe], F32, tag=f"u{key}", name=f"u_{key}{tag}")
                        if s_one:
                            e.tensor_add(out=u[:], in0=pprev, in1=c)
                        else:
                            s_b = s_t.unsqueeze(1).broadcast_to([P, 2, fe])
                            e.tensor_tensor(out=u[:], in0=pprev, in1=s_b, op=mult)
                            e.tensor_add(out=u[:], in0=u[:], in1=c)
                        e.tensor_add(out=out, in0=u[:], in1=m[:])

                # ---- theta pass: t = 1..n-1 computes theta'_{t+1} ----
                for t in range(1, n):
                    for key in ("v", "p"):
                        out = (THn[key][:] if t == n - 1 else th_slot(key, t + 1))
                        advance(key, t, th_slot(key, t), th_slot_swap(key, t),
                                th_slot(key, t - 1), out, sf_t[:, t:t + 1], f"t{t}")

                # ---- w = i / theta'_n -> ring slot 0 (phi''_0) ----
                for key in ("v", "p"):
                    fe = fe_of[key]
                    e = eng[key]
                    dr, di = THn[key][:, 0], THn[key][:, 1]
                    t1 = tmp[key].tile([P, fe], F32, tag=f"w1{key}", name=f"t1{key}")
                    e.tensor_mul(out=t1[:], in0=dr, in1=dr)
                    t2 = tmp[key].tile([P, fe], F32, tag=f"w2{key}", name=f"t2{key}")
                    e.tensor_mul(out=t2[:], in0=di, in1=di)
                    e.tensor_add(out=t1[:], in0=t1[:], in1=t2[:])
                    inv = tmp[key].tile([P, fe], F32, tag=f"w3{key}", name=f"inv{key}")
                    # Pool has no reciprocal; DVE computes it for both slices
                    # (one-time cross-engine handoff at the theta/phi boundary).
                    nc.vector.reciprocal(out=inv[:], in_=t1[:])
                    e.tensor_mul(out=ring[key][:, 0, 0], in0=di, in1=inv[:])
                    e.tensor_mul(out=ring[key][:, 0, 1], in0=dr, in1=inv[:])
                    # phi''_1 = w + A2[n-1] (x) w_swap -> ring slot 1
                    m0 = tmp[key].tile([P, 2, fe], F32, tag=f"m{key}", name=f"m0{key}")
                    e.tensor_tensor(
                        out=m0[:], in0=A2[key][:, :, :, n - 1],
                        in1=ring[key][:, 0, ::-1], op=mult,
                    )
                    e.tensor_add(out=ring[key][:, 1], in0=m0[:], in1=ring[key][:, 0])

                def combine_batch(key, b):
                    """G_{n-1-m} = theta'_{n-1-m} (x) phi''_m, m in [CB*b, CB*b+CB);
                    G rows overwrite the theta rows they consume."""
                    fe = fe_of[key]
                    e = eng[key]
                    m0i = CB * b
                    s0 = m0i % RING
                    hi = (n - 1) - m0i
                    xs = None if hi - CB < 0 else hi - CB
                    X = TH[key][:, :, hi:xs:-1, :].transpose([0, 2, 3, 1])  # (P,CB,2,fe)
                    Y = ring[key][:, s0:s0 + CB]
                    Ys = ring[key][:, s0:s0 + CB, ::-1]
                    q1 = qpool[key].tile([P, CB, 2, fe], F32, tag=f"q1{key}", name=f"q1{key}b{b}")
                    e.tensor_tensor(out=q1[:], in0=X, in1=Y, op=mult)
                    q2 = qpool[key].tile([P, CB, 2, fe], F32, tag=f"q2{key}", name=f"q2{key}b{b}")
                    e.tensor_tensor(out=q2[:], in0=X, in1=Ys, op=mult)
                    og_r = TH[key][:, :, hi:xs:-1, 0].transpose([0, 2, 1])  # (P,CB,fe)
                    og_i = TH[key][:, :, hi:xs:-1, 1].transpose([0, 2, 1])
                    if key == "v":
                        e.scalar_tensor_tensor(
                            out=og_r, in0=q1[:, :, 0, :], scalar=1.0,
                            in1=q1[:, :, 1, :], op0=mult, op1=sub,
                        )
                        e.scalar_tensor_tensor(
                            out=og_i, in0=q2[:, :, 0, :], scalar=1.0,
                            in1=q2[:, :, 1, :], op0=mult, op1=add,
                        )
                    else:
                        e.tensor_sub(out=og_r, in0=q1[:, :, 0, :], in1=q1[:, :, 1, :])
                        e.tensor_add(out=og_i, in0=q2[:, :, 0, :], in1=q2[:, :, 1, :])

                def ring_slot(key, i, swap=False):
                    sl = ring[key][:, i % RING]
                    return ring[key][:, i % RING, ::-1] if swap else sl

                # ---- phi pass: t = 1..n-2 computes phi''_{t+1} ----
                for t in range(1, n - 1):
                    for key in ("v", "p"):
                        advance(key, n - 1 - t, ring_slot(key, t),
                                ring_slot(key, t, swap=True), ring_slot(key, t - 1),
                                ring_slot(key, t + 1), sb_t[:, t:t + 1], f"b{t}")
                        if (t + 2) % CB == 0:
                            combine_batch(key, (t + 2) // CB - 1)

                # ---- G out: TH now holds g rows in (row, k, comp) order ----
                for key in ("v", "p"):
                    fe, j = fe_of[key], j0[key]
                    nc.sync.dma_start(out=g4[:, j:j + fe], in_=TH[key][:])

    nc.compile()
    return nc


def make_aux(h0_diag: np.ndarray, h0_sub: np.ndarray, h0_super: np.ndarray, n: int):
    s = (h0_super * h0_sub).astype(np.float32)          # (n-1,)
    d = h0_diag.astype(np.float32)                      # (n,)
    svf = np.zeros(n, np.float32)
    svf[1:] = s                                          # svf[t] = s[t-1]
    svb = np.zeros(n, np.float32)
    svb[1:n - 1] = s[::-1][:n - 2]                       # svb[t] = s[n-1-t]
    dvec = np.broadcast_to(d, (P, n)).copy()
    svf = np.broadcast_to(svf, (P, n)).copy()
    svb = np.broadcast_to(svb, (P, n)).copy()
    return dvec, svf, svb


def _get_nc(b_core, n, f, n_cores, s_one):
    key = (b_core, n, f, n_cores, s_one)
    if key not in _CACHE:
        _CACHE[key] = build_nc(b_core, n, f, n_cores=n_cores, s_one=s_one)
    return _CACHE[key]


def kernel(he_diag, h0_diag, h0_sub, h0_super):
    from concourse.bass_utils import run_bass_kernel_spmd

    he_diag = np.ascontiguousarray(np.asarray(he_diag, dtype=np.float32))
    B, n = he_diag.shape
    n_cores = 8
    assert B % n_cores == 0
    b_core = B // n_cores
    assert b_core % P == 0
    f = b_core // P

    dvec, svf, svb = make_aux(
        np.asarray(h0_diag), np.asarray(h0_sub), np.asarray(h0_super), n
    )
    s = np.asarray(h0_super, dtype=np.float32) * np.asarray(h0_sub, dtype=np.float32)
    s_one = bool(np.all(s == np.float32(1.0)))
    nc = _get_nc(b_core, n, f, n_cores, s_one)
    in_maps = [
        {"he": he_diag[c * b_core:(c + 1) * b_core],
         "dvec": dvec, "svf": svf, "svb": svb}
        for c in range(n_cores)
    ]
    res = run_bass_kernel_spmd(nc, in_maps, list(range(n_cores)))
    out = np.concatenate(
        [res.results[c]["g"].reshape(b_core, n, 2) for c in range(n_cores)], axis=0
    )
    return out


# revision 9
# speedup vs baseline: 1.1049x; 1.1049x over previous
"""Trainium2 Bass kernel for nn_CUDAOptimizedBKCore: diagonal Green's function
of a complex-shifted tridiagonal matrix via forward/backward continuant
recursions (theta/phi), data-parallel over the batch across 8 NeuronCores.

Self-contained: takes FULL inputs, shards B across cores, runs the Bass
program via run_bass_kernel_spmd, gathers the FULL output.

Per-core design (v3):
  - Row dim f is split into engine-private slices (DVE / Pool); each engine
    runs the whole pipeline on its slice -> no cross-engine deps.
  - k-major layout for the scan: TH[P, n, 2, f] doubles as scan state and
    theta history; every scan op streams contiguous rows.
  - Scan step is 4 TTs: m = a (x) c_swap (a held bf16, which costs ~6e-4
    rel err), u = c + s*p, o_r = u_r - m_r, o_i = u_i + m_i.
  - Combine is batched (CB steps): q1 = X (x) (phi_r, phi_i),
    q2 = X (x) (phi_i, -phi_r), then ONE tensor_sub whose paired access
    pattern emits both G components, writing the j-major G tile in
    ascending 64B runs. G[P, f, n, 2] is per-partition contiguous, so the
    single final DMA moves at ~full HBM rate. he is staged inside G's SBUF
    space (dead until the combines, long after the A2 build consumes he).
"""
import numpy as np

import concourse.bass as bass
import concourse.bacc as bacc
import concourse.tile as tile
from concourse import mybir

F32 = mybir.dt.float32
BF16 = mybir.dt.bfloat16
P = 128
RING = 5
CB = 5          # combine batch (phi'' values per batched combine)

_CACHE = {}


def build_nc(b_core: int, n: int, f: int, n_cores: int = 8, loops: int = 1,
             s_one: bool = True, fv: int = 96):
    """Build the Bacc program for one core's slice (b_core rows, n steps)."""
    assert b_core == P * f
    assert n % CB == 0 and RING == CB
    fp = f - fv                 # Pool row slice
    nc = bacc.Bacc("TRN2", target_bir_lowering=False, debug=False, num_devices=n_cores)
    he = nc.dram_tensor("he", [b_core, n], F32, kind="ExternalInput").ap()
    dvec = nc.dram_tensor("dvec", [P, n], F32, kind="ExternalInput").ap()
    svf = nc.dram_tensor("svf", [P, n], F32, kind="ExternalInput").ap()
    svb = nc.dram_tensor("svb", [P, n], F32, kind="ExternalInput").ap()
    g = nc.dram_tensor("g", [b_core, 2 * n], F32, kind="ExternalOutput").ap()

    mult, add, sub = mybir.AluOpType.mult, mybir.AluOpType.add, mybir.AluOpType.subtract
    he_flat = he.rearrange("(p f) k -> p (f k)", p=P)    # [P, f*n] contiguous
    g4 = g.rearrange("(p f) (k c) -> p f k c", p=P, c=2)

    KC = 16                     # A2 build chunk (k columns per op)
    assert n % KC == 0

    with tile.TileContext(nc) as tc:
        with (
            tc.tile_pool(name="aux", bufs=1) as aux,
            tc.tile_pool(name="big", bufs=1) as big,
            tc.tile_pool(name="tmpv", bufs=1) as tmpv,
            tc.tile_pool(name="tmpp", bufs=1) as tmpp,
        ):
            d_t = aux.tile([P, n], F32)
            nc.sync.dma_start(out=d_t[:], in_=dvec)
            sf_t = aux.tile([P, n], F32)
            nc.sync.dma_start(out=sf_t[:], in_=svf)
            sb_t = aux.tile([P, n], F32)
            nc.sync.dma_start(out=sb_t[:], in_=svb)
            zero_t = aux.tile([P, 1], F32)
            nc.gpsimd.memset(zero_t[:], 0.0)

            import contextlib
            loop_cm = tc.For_i(0, loops, 1) if loops > 1 else contextlib.nullcontext()
            with loop_cm:
                A2 = big.tile([P, n, f], BF16, name="A2")        # +a, k-major
                TH = big.tile([P, n, 2, f], F32, name="TH")      # theta'_0..n-1
                THn = big.tile([P, 2, f], F32, name="THn")       # theta'_n
                ring = big.tile([P, RING, 3, f], F32, name="ring")
                G = big.tile([P, f, n, 2], F32, name="G")        # j-major output
                Q = big.tile([P, 2, CB, 2, f], F32, name="Q")

                j0 = {"v": 0, "p": fv}
                eng = {"v": nc.vector, "p": nc.gpsimd}
                tmp = {"v": tmpv, "p": tmpp}
                fe_of = {"v": fv, "p": fp}
                keys = [k for k in ("v", "p") if fe_of[k] > 0]

                # he staged packed into G's first f*n floats.
                Gf = G[:].rearrange("p f k c -> p (f k c)")
                nc.sync.dma_start(out=Gf[:, 0:f * n], in_=he_flat)
                heS = Gf[:, 0:f * n].rearrange("p (f k) -> p f k", f=f)

                # ---- A2 = bf16(he + d), built in k-chunks per engine ----
                for key in keys:
                    fe, j = fe_of[key], j0[key]
                    e = eng[key]
                    for c0 in range(0, n, KC):
                        out = A2[:, c0:c0 + KC, j:j + fe]
                        hin = heS[:, j:j + fe, c0:c0 + KC].transpose([0, 2, 1])
                        din = d_t[:, c0:c0 + KC].unsqueeze(2).broadcast_to([P, KC, fe])
                        e.tensor_tensor(out=out, in0=hin, in1=din, op=add)

                # ---- init theta'_0 = (1, 0); theta'_1 = (1, a_0) ----
                for key in keys:
                    fe, j = fe_of[key], j0[key]
                    e = eng[key]
                    e.memset(TH[:, 0, 0, j:j + fe], 1.0)
                    e.memset(TH[:, 0, 1, j:j + fe], 0.0)
                    e.memset(TH[:, 1, 0, j:j + fe], 1.0)
                    e.tensor_copy(TH[:, 1, 1, j:j + fe], A2[:, 0, j:j + fe])

                def advance(key, a_k, c, c_swap, pprev, out, s_t):
                    """out = (c + s*p) -+ a (x) c_swap  (4 TTs, one engine)."""
                    fe, j = fe_of[key], j0[key]
                    e = eng[key]
                    ab = A2[:, a_k, j:j + fe].unsqueeze(1).broadcast_to([P, 2, fe])
                    m = tmp[key].tile([P, 2, fe], F32, tag=f"m{key}", name=f"m{key}")
                    e.tensor_tensor(out=m[:], in0=ab, in1=c_swap, op=mult)
                    u = tmp[key].tile([P, 2, fe], F32, tag=f"u{key}", name=f"u{key}")
                    if s_one:
                        e.tensor_add(out=u[:], in0=pprev, in1=c)
                    else:
                        s_b = s_t.unsqueeze(1).broadcast_to([P, 2, fe])
                        e.tensor_tensor(out=u[:], in0=pprev, in1=s_b, op=mult)
                        e.tensor_add(out=u[:], in0=u[:], in1=c)
                    e.tensor_sub(out=out[:, 0], in0=u[:, 0], in1=m[:, 0])
                    e.tensor_add(out=out[:, 1], in0=u[:, 1], in1=m[:, 1])

                # ---- theta pass: t = 1..n-1 computes theta'_{t+1} ----
                for t in range(1, n):
                    for key in keys:
                        fe, j = fe_of[key], j0[key]
                        out = (THn[:, :, j:j + fe] if t == n - 1
                               else TH[:, t + 1, :, j:j + fe])
                        advance(key, t, TH[:, t, :, j:j + fe],
                                TH[:, t, ::-1, j:j + fe],
                                TH[:, t - 1, :, j:j + fe], out, sf_t[:, t:t + 1])

                # ---- w = i / theta'_n -> ring slot 0 ----
                for key in keys:
                    fe, j = fe_of[key], j0[key]
                    e = eng[key]
                    dr, di = THn[:, 0, j:j + fe], THn[:, 1, j:j + fe]
                    t1 = tmp[key].tile([P, fe], F32, tag=f"w1{key}", name=f"t1{key}")
                    e.tensor_mul(out=t1[:], in0=dr, in1=dr)
                    t2 = tmp[key].tile([P, fe], F32, tag=f"w2{key}", name=f"t2{key}")
                    e.tensor_mul(out=t2[:], in0=di, in1=di)
                    e.tensor_add(out=t1[:], in0=t1[:], in1=t2[:])
                    inv = tmp[key].tile([P, fe], F32, tag=f"w3{key}", name=f"inv{key}")
                    # Pool has no reciprocal; DVE computes both slices.
                    nc.vector.reciprocal(out=inv[:], in_=t1[:])
                    e.tensor_mul(out=ring[:, 0, 0, j:j + fe], in0=di, in1=inv[:])
                    e.tensor_mul(out=ring[:, 0, 1, j:j + fe], in0=dr, in1=inv[:])
                    # phi''_1 = w -+ a_{n-1} (x) w_swap -> ring slot 1
                    ab = A2[:, n - 1, j:j + fe].unsqueeze(1).broadcast_to([P, 2, fe])
                    m0 = tmp[key].tile([P, 2, fe], F32, tag=f"m{key}", name=f"m0{key}")
                    e.tensor_tensor(out=m0[:], in0=ab,
                                    in1=ring[:, 0, 1::-1, j:j + fe], op=mult)
                    e.tensor_sub(out=ring[:, 1, 0, j:j + fe],
                                 in0=ring[:, 0, 0, j:j + fe], in1=m0[:, 0])
                    e.tensor_add(out=ring[:, 1, 1, j:j + fe],
                                 in0=ring[:, 0, 1, j:j + fe], in1=m0[:, 1])

                def combine_batch(key, b):
                    """G_{n-1-m} = theta'_{n-1-m} (x) phi''_m, m in [CB*b, CB*b+CB)."""
                    fe, j = fe_of[key], j0[key]
                    e = eng[key]
                    hi = (n - 1) - CB * b
                    lo = hi - CB + 1
                    # ring[s, 2] = -ring[s, 0] for all batch slots (one op)
                    if key == "v":
                        e.tensor_scalar_mul(ring[:, :, 2, j:j + fe],
                                            ring[:, :, 0, j:j + fe], -1.0)
                    else:
                        zb = zero_t.unsqueeze(1).broadcast_to([P, RING, fe])
                        e.tensor_tensor(out=ring[:, :, 2, j:j + fe], in0=zb,
                                        in1=ring[:, :, 0, j:j + fe], op=sub)
                    # X ascending k = lo..hi ; ring slots descending to pair
                    X = TH[:, lo:hi + 1, :, j:j + fe]                 # (P,CB,2,fe)
                    Yr = ring[:, ::-1, 0:2, j:j + fe]
                    Ys = ring[:, ::-1, 1:3, j:j + fe]
                    q1 = Q[:, 0, :, :, j:j + fe]
                    q2 = Q[:, 1, :, :, j:j + fe]
                    e.tensor_tensor(out=q1, in0=X, in1=Yr, op=mult)
                    e.tensor_tensor(out=q2, in0=X, in1=Ys, op=mult)
                    # og: one TT-sub; G's c axis selects q1 (real) vs q2 (imag).
                    qa = Q[:, :, :, 0, j:j + fe].transpose([0, 3, 2, 1])  # (P,fe,CB,2)
                    qb = Q[:, :, :, 1, j:j + fe].transpose([0, 3, 2, 1])
                    og = G[:, j:j + fe, lo:hi + 1, :]                  # (P,fe,CB,2)
                    e.tensor_tensor(out=og, in0=qa, in1=qb, op=sub)

                # ---- phi pass: t = 1..n-2 computes phi''_{t+1} ----
                for t in range(1, n - 1):
                    for key in keys:
                        fe, j = fe_of[key], j0[key]
                        advance(key, n - 1 - t, ring[:, t % RING, 0:2, j:j + fe],
                                ring[:, t % RING, 1::-1, j:j + fe],
                                ring[:, (t - 1) % RING, 0:2, j:j + fe],
                                ring[:, (t + 1) % RING, 0:2, j:j + fe],
                                sb_t[:, t:t + 1])
                        if (t + 2) % CB == 0:
                            combine_batch(key, (t + 2) // CB - 1)

                # ---- G out: one contiguous DMA per core ----
                nc.sync.dma_start(out=g4, in_=G[:])

    nc.compile()
    return nc


def make_aux(h0_diag: np.ndarray, h0_sub: np.ndarray, h0_super: np.ndarray, n: int):
    s = (h0_super * h0_sub).astype(np.float32)          # (n-1,)
    d = h0_diag.astype(np.float32)                      # (n,)
    svf = np.zeros(n, np.float32)
    svf[1:] = s                                          # svf[t] = s[t-1]
    svb = np.zeros(n, np.float32)
    svb[1:n - 1] = s[::-1][:n - 2]                       # svb[t] = s[n-1-t]
    dvec = np.broadcast_to(d, (P, n)).copy()
    svf = np.broadcast_to(svf, (P, n)).copy()
    svb = np.broadcast_to(svb, (P, n)).copy()
    return dvec, svf, svb


def _get_nc(b_core, n, f, n_cores, s_one):
    key = (b_core, n, f, n_cores, s_one)
    if key not in _CACHE:
        _CACHE[key] = build_nc(b_core, n, f, n_cores=n_cores, s_one=s_one)
    return _CACHE[key]


def kernel(he_diag, h0_diag, h0_sub, h0_super):
    from concourse.bass_utils import run_bass_kernel_spmd

    he_diag = np.ascontiguousarray(np.asarray(he_diag, dtype=np.float32))
    B, n = he_diag.shape
    n_cores = 8
    assert B % n_cores == 0
    b_core = B // n_cores
    assert b_core % P == 0
    f = b_core // P

    dvec, svf, svb = make_aux(
        np.asarray(h0_diag), np.asarray(h0_sub), np.asarray(h0_super), n
    )
    s = np.asarray(h0_super, dtype=np.float32) * np.asarray(h0_sub, dtype=np.float32)
    s_one = bool(np.all(s == np.float32(1.0)))
    nc = _get_nc(b_core, n, f, n_cores, s_one)
    in_maps = [
        {"he": he_diag[c * b_core:(c + 1) * b_core],
         "dvec": dvec, "svf": svf, "svb": svb}
        for c in range(n_cores)
    ]
    res = run_bass_kernel_spmd(nc, in_maps, list(range(n_cores)))
    out = np.concatenate(
        [res.results[c]["g"].reshape(b_core, n, 2) for c in range(n_cores)], axis=0
    )
    return out
